# revision 40
# baseline (speedup 1.0000x reference)
"""Trainium2 Bass kernel for the time-binned MoE EmbeddingClassifier.

Model: 11 expert MLPs (1536 -> 3072 -> 3072 -> 5242, exact GELU between
layers, log_softmax output). Each sample is routed to one expert by
bin = trunc((1 - mask_frac) / 0.1).

Strategy (8 NeuronCores, expert-parallel with host-side routing):
  - Routing is computed on the host from mask_frac; samples are grouped by
    expert. Only the routed expert runs per sample (11x less compute than
    the reference's run-all-then-select).
  - Experts 0..7 are whole-expert assigned to cores 0..7.
  - Experts 8 and 9 are each split 4 ways along the hidden dimension
    (cores 0-3 handle expert 8, cores 4-7 handle expert 9): each core
    computes the full layer 1, a 768-column slice of layer 2, and the
    matching 768-row slice of layer 3, producing a full-width partial
    logit sum. The host adds the 4 partials + b3 and applies log_softmax.
  - Precision: everything that streams from HBM is e4m3 fp8 (weights with
    a x64 power-of-2 pre-scale -- |W|~0.02 sits in e4m3's subnormal range
    -- and x/activations unscaled), with fp32 PSUM accumulation and the
    descale folded into the PSUM-drain ACT ops. All three layers run the
    PE's DoubleRow fp8 perf mode (2x rate). Measured rel err ~2.5e-3 vs
    the fp32 reference (gate 2e-2).
  - The kernel is DMA-bound (~42 MB of weights per core at ~360 GB/s), so
    the schedule keeps the weight stream saturated: l1(whole), l1(quarter),
    l2(whole), l2(quarter), then layer 3 of both units chunk-interleaved
    so the tail never bunches. log_softmax is split: the device emits fp16
    logits plus 11 per-chunk exp-sums (the exp-accumulate rides each PSUM
    drain on the ACT engine), and the host finishes the logsumexp and the
    subtract -- this removes the on-device reduction tail and the fp32 z
    buffer. The first weight blocks spread their DMA pieces over two
    queues so the DGE setups overlap at startup, and the tail's D2H
    drains ride three different queues.
  - Expert 10 (hit only when mask_frac == 0.0 exactly) and any samples
    beyond the per-expert capacity of 128 are computed on the host in
    fp32 as a correctness fallback.
  - When all biases are zero (as setup_inputs constructs them) the kernel
    compiles a variant without the per-chunk K=1 bias matmuls; the
    log_softmax pad correction folds into one scalar op (zero-padded W3
    columns give exp(0) = 1 per pad column exactly). Nonzero biases
    compile the general variant.

Device layout: activations ride the partition dim as [samples<=128, feat];
weights stream as the moving matmul operand. Weights are host-packed into
per-output-chunk column blocks ([128, nk*cw] per chunk) so the k-loop
accumulates into a single PSUM bank with back-to-back matmuls, and each
block arrives via ~0.5-1 MB DMA pieces so the PE never starves. Between
layers the activations are transposed 128x128 via the PE.
"""

import os
import sys

if "/opt/trn_rl_repo" not in sys.path:
    sys.path.insert(0, "/opt/trn_rl_repo")

import numpy as np
import ml_dtypes

import concourse.bass as bass
import concourse.tile as tile
from concourse import bacc, mybir
from concourse.bass_utils import run_bass_kernel_spmd

# half dtype for h activations: fp16 beats bf16 here -- same bytes and PE
# rate, but 8x finer mantissa on this small-range data
MOE_HALF = os.environ.get("MOE_HALF", "fp16")
BF16 = mybir.dt.float16 if MOE_HALF == "fp16" else mybir.dt.bfloat16
FP8 = mybir.dt.float8e4
F32 = mybir.dt.float32
AF = mybir.ActivationFunctionType
NBF = np.float16 if MOE_HALF == "fp16" else ml_dtypes.bfloat16
NF8 = ml_dtypes.float8_e4m3
FP8_SCALE = 64.0     # power-of-2 pre-scale: |W|~0.02 sits in e4m3's subnormal
                     # range, x64 recenters it; descale rides the ACT op
# which layers stream fp8 weights ("l1"/"l2"/"l3"); overridable for A/B runs
FP8_LAYERS = frozenset(
    os.environ.get("MOE_FP8", "l1,l2,l3").replace(",", " ").split())
# DoubleRow perf mode for the fp8 layers (needs fp8 activations/x too)
MOE_DR = os.environ.get("MOE_DR", "1") == "1"
DRMODE = mybir.MatmulPerfMode.DoubleRow

E = 11
D = 1536
H = 3072
C = 5242
B = 1024
CAP = 128            # per-expert sample capacity on device
CPAD = 5248          # C padded to a multiple of 128 (10x512 + 128)
CMAIN = 5120         # first 10 layer-3 chunks (512 wide)
NK1 = D // 128       # 12 k-tiles for layer 1
NK2 = H // 128       # 24 k-tiles for layers 2/3
QCOLS = H // 4       # 768-wide hidden slice for the split experts
QSL = CPAD // 4      # 1312-wide C slice each split-expert core emits
PAD_BIAS = -100.0    # b3 value for padded logit columns -> exp() == 0

LAST_RESULTS = None  # BassKernelResults of the most recent run (for test.py)

_NC_CACHE = {}


def _dr_on(layer):
    """DoubleRow applies to a layer when MOE_DR is set and that layer's
    weights (and its lhs activations) are fp8."""
    if not MOE_DR:
        return False
    return layer in FP8_LAYERS


def _wload(nc, wpool, wdram, jrow, cols, npieces, name, wdt=BF16):
    """Fetch one [128, cols] weight block from DRAM row-block jrow into a
    fresh SBUF tile on the SP (sync) HWDGE queue. All weight blocks ride
    this one queue in consumption order; per-shape tags keep big and small
    blocks in separate slot classes so a small late-phase block never waits
    on a big block's slot. npieces==0 is the kernel-open warmup split."""
    pool = wpool[cols] if isinstance(wpool, dict) else wpool
    wblk = pool.tile([128, cols], wdt, tag=f"wblk{cols}", name=f"wb_{name}")
    if npieces == 0:     # warmup split: small leading pieces so the first
        bounds = [0, 1024, 3072, 6144, cols]   # matmuls start ~1 us in
        bounds = sorted(set(min(b, cols) for b in bounds))
    else:
        bounds = [cols * pc // npieces for pc in range(npieces)] + [cols]
    for c0, c1 in zip(bounds, bounds[1:]):
        nc.sync.dma_start(
            wblk[:, c0:c1], wdram[jrow * 128:(jrow + 1) * 128, c0:c1])
    return wblk


def _chunk_mm(nc, pspool, lhs_full, nk, wsb, cw, name, final_stop=False,
              dr=False):
    """Accumulate one [128, cw] output chunk over nk k-tiles into one PSUM
    tile, streaming weights from the SBUF block slice wsb [128, nk*cw]."""
    psum = pspool.tile([128, 512], F32, tag="acc", name=f"ps_{name}")
    if dr:
        for t in range(nk // 2):
            lhs = lhs_full[:, 256 * t:256 * (t + 1)].rearrange(
                "p (i m) -> p i m", i=2)
            rhs = wsb[:, 2 * cw * t:2 * cw * (t + 1)].rearrange(
                "p (i n) -> p i n", i=2)
            nc.tensor.matmul(psum[:, :cw], lhs, rhs, perf_mode=DRMODE,
                             start=(t == 0),
                             stop=(final_stop and t == nk // 2 - 1))
    else:
        for k in range(nk):
            nc.tensor.matmul(psum[:, :cw], lhs_full[:, k * 128:(k + 1) * 128],
                             wsb[:, k * cw:(k + 1) * cw],
                             start=(k == 0),
                             stop=(final_stop and k == nk - 1))
    return psum


def _transpose(nc, hpool, tppool, src, ncols, ident_t, name, hdt=BF16,
               out=None):
    """Transpose src [128, ncols] per 128-chunk -> tile [128, ncols].
    Tensor-engine transpose (bf16) + DVE drain-copy casting to hdt."""
    if out is None:
        out = hpool.tile([128, H], hdt, tag="ht" if hdt is FP8 else "h",
                         name=f"t_{name}")
    for k in range(ncols // 128):
        tp = tppool.tile([128, 128], BF16, tag="tp", name=f"tp_{name}_{k}")
        nc.tensor.transpose(tp[:], src[:, k * 128:(k + 1) * 128], ident_t[:])
        nc.vector.tensor_copy(out[:, k * 128:(k + 1) * 128], tp[:])
    return out


CC_GROUPS = [[0, 1, 2, 3], [4, 5, 6, 7]]


def _exchange(nc, hpool, part, cc_in, cc_out, name, hdt):
    """AllGather the [128, QCOLS] transposed quarter `part` across this
    core's 4-core group via internal DRAM (SBUF -> cc_in -> AllGather ->
    cc_out -> SBUF), concatenating the 4 ranks' k-tile column groups into
    the full [128, H] transposed activation. Rank r in the group owns
    hidden quarter r, so row-block r of the gather lands at cols r*QCOLS."""
    # every hop rides the otherwise-idle Pool engine (SWDGE): a collective
    # wait must never sit in the SP/ACT instruction streams, where it would
    # stall the weight queue or the PSUM drains for its full latency
    nc.gpsimd.dma_start(cc_in, part[:, :QCOLS])
    nc.gpsimd.collective_compute(
        "AllGather", mybir.AluOpType.bypass,
        replica_groups=CC_GROUPS, ins=[cc_in], outs=[cc_out])
    full = hpool.tile([128, H], hdt, tag="ht" if hdt is FP8 else "h",
                      name=f"x_{name}")
    for r in range(4):
        nc.gpsimd.dma_start(full[:, r * QCOLS:(r + 1) * QCOLS],
                            cc_out[r * 128:(r + 1) * 128, :])
    return full


def _unit_front(nc, pools, xs, w1cb, b1s, ones_t, ident_t, uname,
                with_bias=True):
    """Whole-unit layer 1 + transpose: x -> gelu(x @ W1 + b1) -> h1T."""
    hpool, wpool, epool, spool, pspool, tppool = pools
    dt1 = FP8 if "l1" in FP8_LAYERS else BF16
    sc1 = 1.0 / FP8_SCALE if dt1 is FP8 else 1.0
    hdt = FP8 if MOE_DR else BF16

    h1 = hpool.tile([128, H], BF16, tag="h", name=f"h1_{uname}")
    hw = NK1 * 512          # 6144 cols per 512-chunk of layer 1
    for jj in range(H // 1024):
        # layer-1 chunks are fetched as 1.57 MB PAIR blocks (two 512-chunks
        # per DMA) so every weight DMA is big
        wblk = _wload(nc, wpool, w1cb, jj, 2 * hw, 2 if jj == 0 else 1,
                      f"{uname}l1p{jj}", wdt=dt1)
        for i in range(2):
            j = 2 * jj + i
            ps = _chunk_mm(nc, pspool, xs, NK1, wblk[:, i * hw:(i + 1) * hw],
                           512, f"{uname}l1j{j}", final_stop=not with_bias,
                           dr=_dr_on("l1"))
            if with_bias:
                nc.tensor.matmul(ps[:], ones_t[:],
                                 b1s[:, j * 512:(j + 1) * 512],
                                 start=False, stop=True)
            nc.scalar.activation(h1[:, j * 512:(j + 1) * 512], ps[:], AF.Gelu,
                                 scale=sc1)
    return _transpose(nc, hpool, tppool, h1, H, ident_t, f"h1_{uname}",
                      hdt=hdt)


def _qfront(nc, pools, xs, w1qcb, b1s, ones_t, ident_t, with_bias=True):
    """Quarter-unit layer 1: only this core's 768-col slice of W1[q] is
    read (the other 3 slices are computed by the peer cores and merged by
    the h1 AllGather) -> gelu(x @ W1q) -> transposed [128, QCOLS] part."""
    hpool, wpool, epool, spool, pspool, tppool = pools
    dt1 = FP8 if "l1" in FP8_LAYERS else BF16
    sc1 = 1.0 / FP8_SCALE if dt1 is FP8 else 1.0
    hdt = FP8 if MOE_DR else BF16

    h1 = hpool.tile([128, QCOLS], BF16, tag="hq", name="h1_qpart")
    wblk = _wload(nc, wpool, w1qcb, 0, 2 * NK1 * 384, 2, "ql1", wdt=dt1)
    for j in range(2):
        ps = _chunk_mm(nc, pspool, xs, NK1,
                       wblk[:, j * NK1 * 384:(j + 1) * NK1 * 384],
                       384, f"ql1j{j}", final_stop=not with_bias,
                       dr=_dr_on("l1"))
        if with_bias:
            nc.tensor.matmul(ps[:, :384], ones_t[:],
                             b1s[:, j * 384:(j + 1) * 384],
                             start=False, stop=True)
        nc.scalar.activation(h1[:, j * 384:(j + 1) * 384], ps[:, :384],
                             AF.Gelu, scale=sc1)
    part = hpool.tile([128, QCOLS], hdt, tag="hqt", name="h1t_qpart")
    return _transpose(nc, hpool, tppool, h1, QCOLS, ident_t, "h1_qpart",
                      hdt=hdt, out=part)


def _unit_mid(nc, pools, h1t, w2cb, b2s, ones_t, ident_t, ncols2, cw2, uname,
              with_bias=True):
    """Layer 2 + transpose: h1T -> gelu(h1 @ W2 + b2) -> h2T. For the
    quarter unit (ncols2 == QCOLS) the result is the [128, QCOLS] part
    destined for the h2 AllGather."""
    hpool, wpool, epool, spool, pspool, tppool = pools
    dt2 = FP8 if "l2" in FP8_LAYERS else BF16
    sc2 = 1.0 / FP8_SCALE if dt2 is FP8 else 1.0
    hdt = FP8 if MOE_DR else BF16
    quarter = ncols2 == QCOLS

    h2 = hpool.tile([128, ncols2], BF16, tag="hq" if quarter else "h",
                    name=f"h2_{uname}")
    for j in range(ncols2 // cw2):
        wblk = _wload(nc, wpool, w2cb, j, NK2 * cw2, 1,
                      f"{uname}l2j{j}", wdt=dt2)
        ps = _chunk_mm(nc, pspool, h1t, NK2, wblk[:], cw2,
                       f"{uname}l2j{j}", dr=_dr_on("l2"),
                       final_stop=not with_bias)
        if with_bias:
            nc.tensor.matmul(ps[:, :cw2], ones_t[:],
                             b2s[:, j * cw2:(j + 1) * cw2],
                             start=False, stop=True)
        nc.scalar.activation(h2[:, j * cw2:(j + 1) * cw2], ps[:, :cw2],
                             AF.Gelu, scale=sc2)
    out = None
    if quarter:
        out = hpool.tile([128, QCOLS], hdt, tag="hqt", name=f"h2t_{uname}")
    return _transpose(nc, hpool, tppool, h2, ncols2, ident_t, f"h2_{uname}",
                      hdt=hdt, out=out)


def _l3_tail(nc, pools, h2t_w, h2t_q, w3cb, w3cbl, w3qcb, w3qcbl, b3s,
             ones_t, ident_t, outw_ap, outq_ap, outs_ap, with_bias=True):
    """Layer 3 of both units, chunk-interleaved so the weight stream stays
    saturated to the end. Whole unit: each PSUM chunk is drained twice on
    the ACT engine -- once through Exp with a running accumulate (the 11
    per-chunk exp-sums ship to the host, which finishes the logsumexp) and
    once as fp16 logits into an SBUF staging tile; the host does the final
    (z - lse) subtract. Quarter unit: fp16 partial logits staged the same
    way; with the h2 AllGather done, each core emits the EXACT logits for
    its 1312-col slice of C (the host just concatenates + softmaxes).
    Staged logits ship D2H in a few LARGE column-piece DMAs: small per-chunk
    drains generate ~1 KB descriptors whose packets round-robin against the
    weight queue on the same SDMA engines and starve it (measured: weight
    stream drops to <100 GB/s while drains are in flight). Quarter-unit L3
    weight blocks are fetched as 786 KB PAIR blocks for the same reason."""
    hpool, wpool, epool, spool, pspool, tppool = pools
    dt3 = FP8 if "l3" in FP8_LAYERS else BF16
    sc3 = 1.0 / FP8_SCALE if dt3 is FP8 else 1.0
    dr3 = _dr_on("l3")

    nk3q = QCOLS // 128
    qw = nk3q * 512         # 3072 cols per 512-chunk of quarter layer 3
    s = spool.tile([128, 11], F32, tag="s", name="s_w")
    zo = spool.tile([128, CPAD], mybir.dt.float16, tag="zo", name="zo_st")
    zh = spool.tile([128, CPAD], mybir.dt.float16, tag="zh", name="zh_st")
    # D2H piece boundaries (in cols) -> drained once all chunks before the
    # boundary are staged, with deliberately small LAST pieces so the
    # post-compute D2H tail is short (the final piece's transfer +
    # completion receipt sit on the critical path into the kernel epilogue)
    zo_cuts = {3: (0, 2048), 7: (2048, 4096), 9: (4096, 5120),
               10: (5120, CPAD)}
    zh_cuts = {4: (0, 2560), 8: (2560, 4608), 9: (4608, 5120),
               10: (5120, CPAD)}

    qblks = {}
    chunks = [(j, j * 512, 512, j) for j in range(10)]
    chunks.append((10, CMAIN, 128, 0))
    for j, c0, cw, jrow in chunks:
        wdram = w3cb if cw == 512 else w3cbl
        wblk = _wload(nc, wpool, wdram, jrow, NK2 * cw, 1,
                      f"wl3j{j}", wdt=dt3)
        ps = _chunk_mm(nc, pspool, h2t_w, NK2, wblk[:], cw,
                       f"wl3j{j}", final_stop=not with_bias, dr=dr3)
        if with_bias:
            nc.tensor.matmul(ps[:, :cw], ones_t[:], b3s[:, c0:c0 + cw],
                             start=False, stop=True)
        e_scr = epool.tile([128, 512], BF16, tag="e", name=f"e_w_{j}")
        nc.scalar.activation(e_scr[:, :cw], ps[:, :cw], AF.Exp, scale=sc3,
                             accum_out=s[:, j:j + 1])
        # staging copy (descale + fp16 cast) rides the otherwise-idle DVE:
        # keeping the ACT engine to one op per chunk stops the tail from
        # going ACT-bound (which holds PSUM banks and stalls the PE)
        nc.vector.tensor_scalar_mul(zo[:, c0:c0 + cw], ps[:, :cw], sc3)

        if cw == 512:
            jj = j // 2
            if j % 2 == 0:       # fetch the PAIR block covering q-chunks
                qblks[jj] = _wload(nc, wpool, w3qcb, jj, 2 * qw, 1,
                                   f"ql3p{jj}", wdt=dt3)
            qsb = qblks[jj][:, (j % 2) * qw:(j % 2 + 1) * qw]
        else:
            qsb = _wload(nc, wpool, w3qcbl, 0, nk3q * 128, 1,
                         f"ql3j{j}", wdt=dt3)[:]
        psq = _chunk_mm(nc, pspool, h2t_q, nk3q, qsb, cw,
                        f"ql3j{j}", final_stop=True, dr=dr3)
        nc.vector.tensor_scalar_mul(zh[:, c0:c0 + cw], psq[:, :cw], sc3)

        if j in zo_cuts:
            a, b = zo_cuts[j]
            nc.scalar.dma_start(outw_ap[:, a:b], zo[:, a:b])
        if j in zh_cuts:
            a, b = zh_cuts[j]
            nc.scalar.dma_start(outq_ap[:, a:b], zh[:, a:b])

        # PE heartbeat: one dummy transpose per chunk keeps the HAM
        # activity window non-idle during the DMA-bound L3 phase, so the
        # PE's clock gate never drops it to 1.2 GHz (a ~1 us idle gap per
        # chunk is inherent here: 2.6 us of matmul per 3.8 us of DMA)
        hb = tppool.tile([128, 128], BF16, tag="tp", name=f"hb_{j}")
        nc.tensor.transpose(hb[:], ident_t[:], ident_t[:])

    # ship the 11 per-chunk exp-sums; the host computes
    # lse = log(sum(s) + (C - CPAD)) (pad cols contribute exp(0)=1 each in
    # the no-bias variant, ~0 in the bias variant). SP queue: it is idle
    # once the last weight piece has issued.
    nc.sync.dma_start(outs_ap[:, :], s[:])


def _build_nc(with_bias=True):
    nc = bacc.Bacc("TRN2", target_bir_lowering=False, debug=False,
                   num_devices=8)

    def din(name, shape, dt=BF16):
        return nc.dram_tensor(name, shape, dt, kind="ExternalInput").ap()

    xdt = FP8 if MOE_DR and "l1" in FP8_LAYERS else BF16
    xw = din("xw", [128, D], xdt)
    xq = din("xq", [128, D], xdt)
    dt1 = FP8 if "l1" in FP8_LAYERS else BF16
    dt2 = FP8 if "l2" in FP8_LAYERS else BF16
    dt3 = FP8 if "l3" in FP8_LAYERS else BF16
    w1cb = din("w1cb", [3 * 128, 2 * NK1 * 512], dt1)
    w2cb = din("w2cb", [6 * 128, NK2 * 512], dt2)
    w3cb = din("w3cb", [10 * 128, NK2 * 512], dt3)
    w3cbl = din("w3cbl", [128, NK2 * 128], dt3)
    w1qcb = din("w1qcb", [3 * 128, 2 * NK1 * 512], dt1)
    w2qcb = din("w2qcb", [2 * 128, NK2 * 384], dt2)
    w3qcb = din("w3qcb", [5 * 128, 2 * 6 * 512], dt3)
    w3qcbl = din("w3qcbl", [128, 6 * 128], dt3)
    if with_bias:
        b1w = din("b1w", [1, H])
        b2w = din("b2w", [1, H])
        b3w = din("b3w", [1, CPAD])
        b1q = din("b1q", [1, H])
        b2q = din("b2q", [1, QCOLS])
    ones = din("ones", [1, 128])
    ident = din("ident", [128, 128])
    outw = nc.dram_tensor("outw", [128, CPAD], mybir.dt.float16,
                          kind="ExternalOutput").ap()
    outq = nc.dram_tensor("outq", [128, CPAD], mybir.dt.float16,
                          kind="ExternalOutput").ap()
    outs = nc.dram_tensor("outs", [128, 11], F32,
                          kind="ExternalOutput").ap()

    with tile.TileContext(nc) as tc:
        with tc.tile_pool(name="hp", bufs=3) as hpool, \
             tc.tile_pool(name="wp12", bufs=6) as wpool12, \
             tc.tile_pool(name="wp9", bufs=3) as wpool9, \
             tc.tile_pool(name="wp6", bufs=3) as wpool6, \
             tc.tile_pool(name="wp3", bufs=1) as wpool3, \
             tc.tile_pool(name="wp1", bufs=1) as wpool1, \
             tc.tile_pool(name="ep", bufs=2) as epool, \
             tc.tile_pool(name="sp", bufs=1) as spool, \
             tc.tile_pool(name="cp", bufs=1) as cpool, \
             tc.tile_pool(name="ps", bufs=5, space="PSUM") as pspool, \
             tc.tile_pool(name="tp", bufs=3, space="PSUM") as tppool:
            wpool = {12288: wpool12, 9216: wpool9, 6144: wpool6,
                     3072: wpool3, 768: wpool1}
            pools = (hpool, wpool, epool, spool, pspool, tppool)

            # x + consts all ride the ACT HWDGE queue so the SP (sync) queue
            # leads with the first weight piece -- the weight stream is the
            # DMA bottleneck and must start at t=0
            xw_t = cpool.tile([128, D], xdt, tag="xw")
            nc.scalar.dma_start(xw_t[:], xw)
            xq_t = cpool.tile([128, D], xdt, tag="xq")
            nc.scalar.dma_start(xq_t[:], xq)
            ones_t = cpool.tile([1, 128], BF16, tag="ones")
            nc.scalar.dma_start(ones_t[:], ones)
            ident_t = cpool.tile([128, 128], BF16, tag="ident")
            nc.scalar.dma_start(ident_t[:], ident)
            if with_bias:
                b1w_t = cpool.tile([1, H], BF16, tag="b1w")
                nc.scalar.dma_start(b1w_t[:], b1w)
                b2w_t = cpool.tile([1, H], BF16, tag="b2w")
                nc.scalar.dma_start(b2w_t[:], b2w)
                b3w_t = cpool.tile([1, CPAD], BF16, tag="b3w")
                nc.scalar.dma_start(b3w_t[:], b3w)
                b1q_t = cpool.tile([1, H], BF16, tag="b1q")
                nc.scalar.dma_start(b1q_t[:], b1q)
                b2q_t = cpool.tile([1, QCOLS], BF16, tag="b2q")
                nc.scalar.dma_start(b2q_t[:], b2q)
            else:
                b1w_t = b2w_t = b3w_t = b1q_t = b2q_t = None

            # quarter layer 1 FIRST (smallest stream prefix) so the h1
            # AllGather across the 4-core group starts as early as possible
            # and its latency hides under the whole unit's weight stream
            h1t_w = _unit_front(nc, pools, xw_t[:], w1cb, b1w_t, ones_t,
                                ident_t, "w", with_bias=with_bias)
            h1t_q = _unit_front(nc, pools, xq_t[:], w1qcb, b1q_t, ones_t,
                                ident_t, "q", with_bias=with_bias)
            h2t_w = _unit_mid(nc, pools, h1t_w, w2cb, b2w_t, ones_t, ident_t,
                              H, 512, "w", with_bias=with_bias)
            h2t_q = _unit_mid(nc, pools, h1t_q, w2qcb, b2q_t, ones_t,
                              ident_t, QCOLS, 384, "q", with_bias=with_bias)
            _l3_tail(nc, pools, h2t_w, h2t_q, w3cb, w3cbl, w3qcb, w3qcbl,
                     b3w_t, ones_t, ident_t, outw, outq, outs,
                     with_bias=with_bias)
    nc.compile()
    return nc


def _cb_pack(W, cw, layer):
    """[K, Ctot] -> per-cw-chunk column blocks [nch*128, nk*cw] where
    block row p, col k*cw + c = W[k*128 + p, j*cw + c]. In DoubleRow mode
    rows pair up per 256-super: col t*2cw + i*cw + c maps to
    row 256t + 128i + p."""
    K, Ct = W.shape
    nk, nch = K // 128, Ct // cw
    if layer in FP8_LAYERS:
        ndt = NF8
        Wr = (np.asarray(W, dtype=np.float32) * FP8_SCALE).astype(NF8)
    else:
        ndt = NBF
        Wr = np.asarray(W, dtype=NBF)
    Wr = Wr.reshape(nk, 128, Ct)
    out = np.empty((nch * 128, nk * cw), dtype=ndt)
    for j in range(nch):
        blk = Wr[:, :, j * cw:(j + 1) * cw]        # [nk, 128, cw]
        if _dr_on(layer):
            # [t, i, p, c] -> [p, t, i, c] -> cols ordered t*2cw + i*cw + c
            out[j * 128:(j + 1) * 128] = (
                blk.reshape(nk // 2, 2, 128, cw).transpose(2, 0, 1, 3)
                .reshape(128, nk * cw))
        else:
            out[j * 128:(j + 1) * 128] = (
                blk.transpose(1, 0, 2).reshape(128, nk * cw))
    return out


def _pair_fold(cb, nblocks):
    """Stack consecutive 128-row blocks side by side: [n*128, w] ->
    [(n//2)*128, 2w], so one DMA fetches two chunks' weights."""
    n128, w = cb.shape
    assert n128 == nblocks * 128 and nblocks % 2 == 0
    out = np.empty((nblocks // 2 * 128, 2 * w), dtype=cb.dtype)
    for jj in range(nblocks // 2):
        out[jj * 128:(jj + 1) * 128, :w] = cb[2 * jj * 128:(2 * jj + 1) * 128]
        out[jj * 128:(jj + 1) * 128, w:] = cb[(2 * jj + 1) * 128:
                                              (2 * jj + 2) * 128]
    return out


def _erf(v):
    try:
        from scipy.special import erf
        return erf(v)
    except ImportError:
        import math
        return np.vectorize(math.erf)(v)


def _host_expert(x_rows, W1e, b1e, W2e, b2e, W3e, b3e):
    """fp32 numpy fallback, mirroring the reference exactly."""

    def gelu(v):
        return (v * 0.5 * (1.0 + _erf(v / np.sqrt(2.0)))).astype(np.float32)

    h1 = gelu(x_rows @ W1e + b1e)
    h2 = gelu(h1 @ W2e + b2e)
    z = (h2 @ W3e + b3e).astype(np.float64)
    m = z.max(axis=1, keepdims=True)
    lse = np.log(np.exp(z - m).sum(axis=1, keepdims=True)) + m
    return (z - lse).astype(np.float32)


def kernel(x, mask_frac, W1, b1, W2, b2, W3, b3):
    global LAST_RESULTS, _NC_CACHE

    x = np.asarray(x, dtype=np.float32)
    mask_frac = np.asarray(mask_frac, dtype=np.float32)
    W1 = np.asarray(W1, dtype=np.float32)
    b1 = np.asarray(b1, dtype=np.float32)
    W2 = np.asarray(W2, dtype=np.float32)
    b2 = np.asarray(b2, dtype=np.float32)
    W3 = np.asarray(W3, dtype=np.float32)
    b3 = np.asarray(b3, dtype=np.float32)

    # host routing, mirroring the reference's fp32 arithmetic
    t = np.float32(1.0) - mask_frac
    bins = (t / np.float32(0.1)).astype(np.int32)

    with_bias = bool(b1.any() or b2.any() or b3.any())

    groups = [np.where(bins == e)[0] for e in range(E)]
    fallback = []  # (expert, sample indices) pairs computed on host
    dev_groups = []
    for e in range(10):
        idx = groups[e]
        if len(idx) > CAP:
            fallback.append((e, idx[CAP:]))
            idx = idx[:CAP]
        dev_groups.append(idx)
    if len(groups[10]):
        fallback.append((10, groups[10]))

    NX = NF8 if (MOE_DR and "l1" in FP8_LAYERS) else NBF

    def pack_x(idx):
        # [128, D] with xs[p, k*128 + n] = x[idx[n], k*128 + p]; this
        # layout is already DoubleRow-compatible (k-tile pairs sit in
        # adjacent 128-col groups)
        xt = np.zeros((128, D), dtype=NX)
        if len(idx):
            xe = x[idx].astype(NX)             # [n, D]
            xr = np.ascontiguousarray(
                xe.reshape(len(idx), NK1, 128).transpose(2, 1, 0))
            xt.reshape(128, NK1, 128)[:, :, :len(idx)] = xr
        return xt

    bsc1 = FP8_SCALE if "l1" in FP8_LAYERS else 1.0
    bsc2 = FP8_SCALE if "l2" in FP8_LAYERS else 1.0
    bsc3 = FP8_SCALE if "l3" in FP8_LAYERS else 1.0
    b3pad = np.full((1, CPAD), PAD_BIAS * bsc3, dtype=NBF)
    ones_np = np.ones((1, 128), dtype=NBF)
    ident_np = np.eye(128, dtype=NBF)

    in_maps = []
    for c in range(8):
        q = 8 if c < 4 else 9          # split expert handled by this core
        qq = c % 4                     # hidden-dim quarter index
        b3row = b3pad.copy()
        b3row[0, :C] = (b3[c] * bsc3).astype(NBF)
        w3pad = np.zeros((H, CPAD), dtype=np.float32)
        w3pad[:, :C] = W3[c]
        w3qpad = np.zeros((QCOLS, CPAD), dtype=np.float32)
        w3qpad[:, :C] = W3[q][qq * QCOLS:(qq + 1) * QCOLS]
        bias_ins = {
            "b1w": (b1[c] * bsc1).astype(NBF).reshape(1, H),
            "b2w": (b2[c] * bsc2).astype(NBF).reshape(1, H),
            "b3w": b3row,
            "b1q": (b1[q] * bsc1).astype(NBF).reshape(1, H),
            "b2q": np.ascontiguousarray(
                (b2[q][qq * QCOLS:(qq + 1) * QCOLS] * bsc2).astype(NBF)
            ).reshape(1, QCOLS),
        } if with_bias else {}
        in_maps.append({
            **bias_ins,
            "xw": pack_x(dev_groups[c]),
            "xq": pack_x(dev_groups[q]),
            "w1cb": _pair_fold(_cb_pack(W1[c], 512, "l1"), 6),
            "w2cb": _cb_pack(W2[c], 512, "l2"),
            "w3cb": _cb_pack(w3pad[:, :CMAIN], 512, "l3"),
            "w3cbl": _cb_pack(w3pad[:, CMAIN:], 128, "l3"),
            "w1qcb": _pair_fold(_cb_pack(W1[q], 512, "l1"), 6),
            "w2qcb": _cb_pack(W2[q][:, qq * QCOLS:(qq + 1) * QCOLS], 384, "l2"),
            "w3qcb": _pair_fold(_cb_pack(w3qpad[:, :CMAIN], 512, "l3"), 10),
            "w3qcbl": _cb_pack(w3qpad[:, CMAIN:], 128, "l3"),
            "ones": ones_np,
            "ident": ident_np,
        })

    if with_bias not in _NC_CACHE:
        _NC_CACHE[with_bias] = _build_nc(with_bias)
    res = run_bass_kernel_spmd(_NC_CACHE[with_bias], in_maps,
                               core_ids=list(range(8)))
    LAST_RESULTS = res

    out = np.zeros((B, C), dtype=np.float32)
    for c in range(8):
        idx = dev_groups[c]
        if len(idx):
            z = res.results[c]["outw"][:len(idx), :C].astype(np.float32)
            stot = (res.results[c]["outs"][:len(idx)]
                    .astype(np.float64).sum(axis=1, keepdims=True))
            lse = np.log(stot + (C - CPAD) * (0.0 if with_bias else 1.0))
            out[idx] = z - lse.astype(np.float32)

    # split experts: host-sum the 4 hidden-quarter partials + b3, log_softmax
    for q, cores in ((8, (0, 1, 2, 3)), (9, (4, 5, 6, 7))):
        idx = dev_groups[q]
        if not len(idx):
            continue
        zsum = np.zeros((len(idx), C), dtype=np.float64)
        for c in cores:
            zsum += res.results[c]["outq"][:len(idx), :C]
        zsum += b3[q]
        m = zsum.max(axis=1, keepdims=True)
        lse = np.log(np.exp(zsum - m).sum(axis=1, keepdims=True)) + m
        out[idx] = (zsum - lse).astype(np.float32)

    for e, idx in fallback:
        out[idx] = _host_expert(x[idx], W1[e], b1[e], W2[e], b2[e],
                                W3[e], b3[e])
    return out



# revision 45
# speedup vs baseline: 1.0010x; 1.0010x over previous
"""Trainium2 Bass kernel for the time-binned MoE EmbeddingClassifier.

Model: 11 expert MLPs (1536 -> 3072 -> 3072 -> 5242, exact GELU between
layers, log_softmax output). Each sample is routed to one expert by
bin = trunc((1 - mask_frac) / 0.1).

Strategy (8 NeuronCores, expert-parallel with host-side routing):
  - Routing is computed on the host from mask_frac; samples are grouped by
    expert. Only the routed expert runs per sample (11x less compute than
    the reference's run-all-then-select).
  - Experts 0..7 are whole-expert assigned to cores 0..7.
  - Experts 8 and 9 are each split 4 ways along the hidden dimension
    (cores 0-3 handle expert 8, cores 4-7 handle expert 9): each core
    computes the full layer 1, a 768-column slice of layer 2, and the
    matching 768-row slice of layer 3, producing a full-width partial
    logit sum. The host adds the 4 partials + b3 and applies log_softmax.
  - Precision: everything that streams from HBM is e4m3 fp8 (weights with
    a x64 power-of-2 pre-scale -- |W|~0.02 sits in e4m3's subnormal range
    -- and x/activations unscaled), with fp32 PSUM accumulation and the
    descale folded into the PSUM-drain ACT ops. All three layers run the
    PE's DoubleRow fp8 perf mode (2x rate). Measured rel err ~2.5e-3 vs
    the fp32 reference (gate 2e-2).
  - The kernel is DMA-bound (~42 MB of weights per core at ~360-430 GB/s
    depending on HBM-stack contention with the neighbor core), so the
    schedule keeps the weight stream saturated: every weight block rides
    the SP HWDGE queue in exact consumption order as a 0.8-1.6 MB DMA
    (layer-1 and quarter-layer-3 chunks are host-packed into PAIR blocks),
    with per-shape SBUF slot pools so small late blocks never wait on big
    blocks' slots. log_softmax is split: the device emits fp16 logits plus
    11 per-chunk exp-sums (the exp-accumulate rides each PSUM drain on the
    ACT engine), and the host finishes the logsumexp and the subtract.
    Logits are STAGED in SBUF and shipped D2H as a few large column pieces
    on the ACT HWDGE -- per-chunk drains would emit ~1 KB descriptors whose
    packets round-robin against the weight queue inside the SDMA engines
    and starve it. The staging copies (descale + fp16 cast) ride the
    otherwise-idle DVE so the ACT engine never gates PSUM-bank recycling.
    A dummy PE transpose after every chunk (all layers) keeps the HAM
    activity window non-idle so the tensor engine is never clock-gated to
    1.2 GHz during DMA-bound stretches or phase seams (measured: cold
    634 ns vs warm 379 ns per matmul; with the heartbeat, zero cold
    matmuls in traces and a gap-free DMA stream even under HBM-stack
    contention from the neighbor core).
    Cross-core collectives were measured unusable here (~70 us for a 98 KB
    4-rank AllGather), so the W1 duplication of the split experts stays.
  - Expert 10 (hit only when mask_frac == 0.0 exactly) and any samples
    beyond the per-expert capacity of 128 are computed on the host in
    fp32 as a correctness fallback.
  - When all biases are zero (as setup_inputs constructs them) the kernel
    compiles a variant without the per-chunk K=1 bias matmuls; the
    log_softmax pad correction folds into one scalar op (zero-padded W3
    columns give exp(0) = 1 per pad column exactly). Nonzero biases
    compile the general variant.

Device layout: activations ride the partition dim as [samples<=128, feat];
weights stream as the moving matmul operand. Weights are host-packed into
per-output-chunk column blocks ([128, nk*cw] per chunk) so the k-loop
accumulates into a single PSUM bank with back-to-back matmuls, and each
block arrives via ~0.5-1 MB DMA pieces so the PE never starves. Between
layers the activations are transposed 128x128 via the PE.
"""

import os
import sys

if "/opt/trn_rl_repo" not in sys.path:
    sys.path.insert(0, "/opt/trn_rl_repo")

import numpy as np
import ml_dtypes

import concourse.bass as bass
import concourse.tile as tile
from concourse import bacc, mybir
from concourse.bass_utils import run_bass_kernel_spmd

# half dtype for h activations: fp16 beats bf16 here -- same bytes and PE
# rate, but 8x finer mantissa on this small-range data
MOE_HALF = os.environ.get("MOE_HALF", "fp16")
BF16 = mybir.dt.float16 if MOE_HALF == "fp16" else mybir.dt.bfloat16
FP8 = mybir.dt.float8e4
F32 = mybir.dt.float32
AF = mybir.ActivationFunctionType
NBF = np.float16 if MOE_HALF == "fp16" else ml_dtypes.bfloat16
NF8 = ml_dtypes.float8_e4m3
FP8_SCALE = 64.0     # power-of-2 pre-scale: |W|~0.02 sits in e4m3's subnormal
                     # range, x64 recenters it; descale rides the ACT op
# which layers stream fp8 weights ("l1"/"l2"/"l3"); overridable for A/B runs
FP8_LAYERS = frozenset(
    os.environ.get("MOE_FP8", "l1,l2,l3").replace(",", " ").split())
# DoubleRow perf mode for the fp8 layers (needs fp8 activations/x too)
MOE_DR = os.environ.get("MOE_DR", "1") == "1"
DRMODE = mybir.MatmulPerfMode.DoubleRow

E = 11
D = 1536
H = 3072
C = 5242
B = 1024
CAP = 128            # per-expert sample capacity on device
CPAD = 5248          # C padded to a multiple of 128 (10x512 + 128)
CMAIN = 5120         # first 10 layer-3 chunks (512 wide)
NK1 = D // 128       # 12 k-tiles for layer 1
NK2 = H // 128       # 24 k-tiles for layers 2/3
QCOLS = H // 4       # 768-wide hidden slice for the split experts
QSL = CPAD // 4      # 1312-wide C slice each split-expert core emits
PAD_BIAS = -100.0    # b3 value for padded logit columns -> exp() == 0

LAST_RESULTS = None  # BassKernelResults of the most recent run (for test.py)

_NC_CACHE = {}


def _dr_on(layer):
    """DoubleRow applies to a layer when MOE_DR is set and that layer's
    weights (and its lhs activations) are fp8."""
    if not MOE_DR:
        return False
    return layer in FP8_LAYERS


def _wload(nc, wpool, wdram, jrow, cols, npieces, name, wdt=BF16):
    """Fetch one [128, cols] weight block from DRAM row-block jrow into a
    fresh SBUF tile on the SP (sync) HWDGE queue. All weight blocks ride
    this one queue in consumption order; per-shape tags keep big and small
    blocks in separate slot classes so a small late-phase block never waits
    on a big block's slot. npieces==0 is the kernel-open warmup split."""
    pool = wpool[cols] if isinstance(wpool, dict) else wpool
    wblk = pool.tile([128, cols], wdt, tag=f"wblk{cols}", name=f"wb_{name}")
    if npieces == 0:     # warmup split: small leading pieces so the first
        bounds = [0, 1024, 3072, 6144, cols]   # matmuls start ~1 us in
        bounds = sorted(set(min(b, cols) for b in bounds))
    else:
        bounds = [cols * pc // npieces for pc in range(npieces)] + [cols]
    for c0, c1 in zip(bounds, bounds[1:]):
        nc.sync.dma_start(
            wblk[:, c0:c1], wdram[jrow * 128:(jrow + 1) * 128, c0:c1])
    return wblk


def _hb(nc, tppool, ident_t, name):
    """Dummy PE transpose: keeps the tensor engine's HAM activity window
    non-idle through DMA-bound stretches and phase seams, so the clock
    gate never drops the PE to 1.2 GHz (cold matmuls cost ~1.7x)."""
    hb = tppool.tile([128, 128], BF16, tag="tp", name=f"hb_{name}")
    nc.tensor.transpose(hb[:], ident_t[:], ident_t[:])


def _chunk_mm(nc, pspool, lhs_full, nk, wsb, cw, name, final_stop=False,
              dr=False):
    """Accumulate one [128, cw] output chunk over nk k-tiles into one PSUM
    tile, streaming weights from the SBUF block slice wsb [128, nk*cw]."""
    psum = pspool.tile([128, 512], F32, tag="acc", name=f"ps_{name}")
    if dr:
        for t in range(nk // 2):
            lhs = lhs_full[:, 256 * t:256 * (t + 1)].rearrange(
                "p (i m) -> p i m", i=2)
            rhs = wsb[:, 2 * cw * t:2 * cw * (t + 1)].rearrange(
                "p (i n) -> p i n", i=2)
            nc.tensor.matmul(psum[:, :cw], lhs, rhs, perf_mode=DRMODE,
                             start=(t == 0),
                             stop=(final_stop and t == nk // 2 - 1))
    else:
        for k in range(nk):
            nc.tensor.matmul(psum[:, :cw], lhs_full[:, k * 128:(k + 1) * 128],
                             wsb[:, k * cw:(k + 1) * cw],
                             start=(k == 0),
                             stop=(final_stop and k == nk - 1))
    return psum


def _transpose(nc, hpool, tppool, src, ncols, ident_t, name, hdt=BF16,
               out=None):
    """Transpose src [128, ncols] per 128-chunk -> tile [128, ncols].
    Tensor-engine transpose (bf16) + DVE drain-copy casting to hdt."""
    if out is None:
        out = hpool.tile([128, H], hdt, tag="ht" if hdt is FP8 else "h",
                         name=f"t_{name}")
    for k in range(ncols // 128):
        tp = tppool.tile([128, 128], BF16, tag="tp", name=f"tp_{name}_{k}")
        nc.tensor.transpose(tp[:], src[:, k * 128:(k + 1) * 128], ident_t[:])
        nc.vector.tensor_copy(out[:, k * 128:(k + 1) * 128], tp[:])
    return out


CC_GROUPS = [[0, 1, 2, 3], [4, 5, 6, 7]]


def _exchange(nc, hpool, part, cc_in, cc_out, name, hdt):
    """AllGather the [128, QCOLS] transposed quarter `part` across this
    core's 4-core group via internal DRAM (SBUF -> cc_in -> AllGather ->
    cc_out -> SBUF), concatenating the 4 ranks' k-tile column groups into
    the full [128, H] transposed activation. Rank r in the group owns
    hidden quarter r, so row-block r of the gather lands at cols r*QCOLS."""
    # every hop rides the otherwise-idle Pool engine (SWDGE): a collective
    # wait must never sit in the SP/ACT instruction streams, where it would
    # stall the weight queue or the PSUM drains for its full latency
    nc.gpsimd.dma_start(cc_in, part[:, :QCOLS])
    nc.gpsimd.collective_compute(
        "AllGather", mybir.AluOpType.bypass,
        replica_groups=CC_GROUPS, ins=[cc_in], outs=[cc_out])
    full = hpool.tile([128, H], hdt, tag="ht" if hdt is FP8 else "h",
                      name=f"x_{name}")
    for r in range(4):
        nc.gpsimd.dma_start(full[:, r * QCOLS:(r + 1) * QCOLS],
                            cc_out[r * 128:(r + 1) * 128, :])
    return full


def _unit_front(nc, pools, xs, w1cb, b1s, ones_t, ident_t, uname,
                with_bias=True):
    """Whole-unit layer 1 + transpose: x -> gelu(x @ W1 + b1) -> h1T."""
    hpool, wpool, epool, spool, pspool, tppool = pools
    dt1 = FP8 if "l1" in FP8_LAYERS else BF16
    sc1 = 1.0 / FP8_SCALE if dt1 is FP8 else 1.0
    hdt = FP8 if MOE_DR else BF16

    h1 = hpool.tile([128, H], BF16, tag="h", name=f"h1_{uname}")
    hw = NK1 * 512          # 6144 cols per 512-chunk of layer 1
    for jj in range(H // 1024):
        # layer-1 chunks are fetched as 1.57 MB PAIR blocks (two 512-chunks
        # per DMA) so every weight DMA is big
        wblk = _wload(nc, wpool, w1cb, jj, 2 * hw, 2 if jj == 0 else 1,
                      f"{uname}l1p{jj}", wdt=dt1)
        for i in range(2):
            j = 2 * jj + i
            ps = _chunk_mm(nc, pspool, xs, NK1, wblk[:, i * hw:(i + 1) * hw],
                           512, f"{uname}l1j{j}", final_stop=not with_bias,
                           dr=_dr_on("l1"))
            if with_bias:
                nc.tensor.matmul(ps[:], ones_t[:],
                                 b1s[:, j * 512:(j + 1) * 512],
                                 start=False, stop=True)
            nc.scalar.activation(h1[:, j * 512:(j + 1) * 512], ps[:], AF.Gelu,
                                 scale=sc1)
        _hb(nc, tppool, ident_t, f"l1{uname}_{jj}")
    return _transpose(nc, hpool, tppool, h1, H, ident_t, f"h1_{uname}",
                      hdt=hdt)


def _qfront(nc, pools, xs, w1qcb, b1s, ones_t, ident_t, with_bias=True):
    """Quarter-unit layer 1: only this core's 768-col slice of W1[q] is
    read (the other 3 slices are computed by the peer cores and merged by
    the h1 AllGather) -> gelu(x @ W1q) -> transposed [128, QCOLS] part."""
    hpool, wpool, epool, spool, pspool, tppool = pools
    dt1 = FP8 if "l1" in FP8_LAYERS else BF16
    sc1 = 1.0 / FP8_SCALE if dt1 is FP8 else 1.0
    hdt = FP8 if MOE_DR else BF16

    h1 = hpool.tile([128, QCOLS], BF16, tag="hq", name="h1_qpart")
    wblk = _wload(nc, wpool, w1qcb, 0, 2 * NK1 * 384, 2, "ql1", wdt=dt1)
    for j in range(2):
        ps = _chunk_mm(nc, pspool, xs, NK1,
                       wblk[:, j * NK1 * 384:(j + 1) * NK1 * 384],
                       384, f"ql1j{j}", final_stop=not with_bias,
                       dr=_dr_on("l1"))
        if with_bias:
            nc.tensor.matmul(ps[:, :384], ones_t[:],
                             b1s[:, j * 384:(j + 1) * 384],
                             start=False, stop=True)
        nc.scalar.activation(h1[:, j * 384:(j + 1) * 384], ps[:, :384],
                             AF.Gelu, scale=sc1)
    part = hpool.tile([128, QCOLS], hdt, tag="hqt", name="h1t_qpart")
    return _transpose(nc, hpool, tppool, h1, QCOLS, ident_t, "h1_qpart",
                      hdt=hdt, out=part)


def _unit_mid(nc, pools, h1t, w2cb, b2s, ones_t, ident_t, ncols2, cw2, uname,
              with_bias=True):
    """Layer 2 + transpose: h1T -> gelu(h1 @ W2 + b2) -> h2T. For the
    quarter unit (ncols2 == QCOLS) the result is the [128, QCOLS] part
    destined for the h2 AllGather."""
    hpool, wpool, epool, spool, pspool, tppool = pools
    dt2 = FP8 if "l2" in FP8_LAYERS else BF16
    sc2 = 1.0 / FP8_SCALE if dt2 is FP8 else 1.0
    hdt = FP8 if MOE_DR else BF16
    quarter = ncols2 == QCOLS

    h2 = hpool.tile([128, ncols2], BF16, tag="hq" if quarter else "h",
                    name=f"h2_{uname}")
    for j in range(ncols2 // cw2):
        wblk = _wload(nc, wpool, w2cb, j, NK2 * cw2, 1,
                      f"{uname}l2j{j}", wdt=dt2)
        ps = _chunk_mm(nc, pspool, h1t, NK2, wblk[:], cw2,
                       f"{uname}l2j{j}", dr=_dr_on("l2"),
                       final_stop=not with_bias)
        if with_bias:
            nc.tensor.matmul(ps[:, :cw2], ones_t[:],
                             b2s[:, j * cw2:(j + 1) * cw2],
                             start=False, stop=True)
        nc.scalar.activation(h2[:, j * cw2:(j + 1) * cw2], ps[:, :cw2],
                             AF.Gelu, scale=sc2)
        _hb(nc, tppool, ident_t, f"l2{uname}_{j}")
    out = None
    if quarter:
        out = hpool.tile([128, QCOLS], hdt, tag="hqt", name=f"h2t_{uname}")
    return _transpose(nc, hpool, tppool, h2, ncols2, ident_t, f"h2_{uname}",
                      hdt=hdt, out=out)


def _l3_tail(nc, pools, h2t_w, h2t_q, w3cb, w3cbl, w3qcb, w3qcbl, b3s,
             ones_t, ident_t, outw_ap, outq_ap, outs_ap, with_bias=True):
    """Layer 3 of both units, chunk-interleaved so the weight stream stays
    saturated to the end. Whole unit: each PSUM chunk is drained twice on
    the ACT engine -- once through Exp with a running accumulate (the 11
    per-chunk exp-sums ship to the host, which finishes the logsumexp) and
    once as fp16 logits into an SBUF staging tile; the host does the final
    (z - lse) subtract. Quarter unit: fp16 partial logits staged the same
    way; with the h2 AllGather done, each core emits the EXACT logits for
    its 1312-col slice of C (the host just concatenates + softmaxes).
    Staged logits ship D2H in a few LARGE column-piece DMAs: small per-chunk
    drains generate ~1 KB descriptors whose packets round-robin against the
    weight queue on the same SDMA engines and starve it (measured: weight
    stream drops to <100 GB/s while drains are in flight). Quarter-unit L3
    weight blocks are fetched as 786 KB PAIR blocks for the same reason."""
    hpool, wpool, epool, spool, pspool, tppool = pools
    dt3 = FP8 if "l3" in FP8_LAYERS else BF16
    sc3 = 1.0 / FP8_SCALE if dt3 is FP8 else 1.0
    dr3 = _dr_on("l3")

    nk3q = QCOLS // 128
    qw = nk3q * 512         # 3072 cols per 512-chunk of quarter layer 3
    s = spool.tile([128, 11], F32, tag="s", name="s_w")
    zo = spool.tile([128, CPAD], mybir.dt.float16, tag="zo", name="zo_st")
    zh = spool.tile([128, CPAD], mybir.dt.float16, tag="zh", name="zh_st")
    # D2H piece boundaries (in cols) -> drained once all chunks before the
    # boundary are staged, with deliberately small LAST pieces so the
    # post-compute D2H tail is short (the final piece's transfer +
    # completion receipt sit on the critical path into the kernel epilogue)
    zo_cuts = {3: (0, 2048), 7: (2048, 4096), 9: (4096, 5120),
               10: (5120, CPAD)}
    zh_cuts = {4: (0, 2560), 8: (2560, 4608), 9: (4608, 5120),
               10: (5120, CPAD)}

    qblks = {}
    chunks = [(j, j * 512, 512, j) for j in range(10)]
    chunks.append((10, CMAIN, 128, 0))
    for j, c0, cw, jrow in chunks:
        wdram = w3cb if cw == 512 else w3cbl
        wblk = _wload(nc, wpool, wdram, jrow, NK2 * cw, 1,
                      f"wl3j{j}", wdt=dt3)
        ps = _chunk_mm(nc, pspool, h2t_w, NK2, wblk[:], cw,
                       f"wl3j{j}", final_stop=not with_bias, dr=dr3)
        if with_bias:
            nc.tensor.matmul(ps[:, :cw], ones_t[:], b3s[:, c0:c0 + cw],
                             start=False, stop=True)
        e_scr = epool.tile([128, 512], BF16, tag="e", name=f"e_w_{j}")
        nc.scalar.activation(e_scr[:, :cw], ps[:, :cw], AF.Exp, scale=sc3,
                             accum_out=s[:, j:j + 1])
        # staging copy (descale + fp16 cast) rides the otherwise-idle DVE:
        # keeping the ACT engine to one op per chunk stops the tail from
        # going ACT-bound (which holds PSUM banks and stalls the PE)
        nc.vector.tensor_scalar_mul(zo[:, c0:c0 + cw], ps[:, :cw], sc3)

        if cw == 512:
            jj = j // 2
            if j % 2 == 0:       # fetch the PAIR block covering q-chunks
                qblks[jj] = _wload(nc, wpool, w3qcb, jj, 2 * qw, 1,
                                   f"ql3p{jj}", wdt=dt3)
            qsb = qblks[jj][:, (j % 2) * qw:(j % 2 + 1) * qw]
        else:
            qsb = _wload(nc, wpool, w3qcbl, 0, nk3q * 128, 1,
                         f"ql3j{j}", wdt=dt3)[:]
        psq = _chunk_mm(nc, pspool, h2t_q, nk3q, qsb, cw,
                        f"ql3j{j}", final_stop=True, dr=dr3)
        nc.vector.tensor_scalar_mul(zh[:, c0:c0 + cw], psq[:, :cw], sc3)

        if j in zo_cuts:
            a, b = zo_cuts[j]
            nc.scalar.dma_start(outw_ap[:, a:b], zo[:, a:b])
        if j in zh_cuts:
            a, b = zh_cuts[j]
            nc.scalar.dma_start(outq_ap[:, a:b], zh[:, a:b])

        _hb(nc, tppool, ident_t, f"l3_{j}")

    # ship the 11 per-chunk exp-sums; the host computes
    # lse = log(sum(s) + (C - CPAD)) (pad cols contribute exp(0)=1 each in
    # the no-bias variant, ~0 in the bias variant). SP queue: it is idle
    # once the last weight piece has issued.
    nc.sync.dma_start(outs_ap[:, :], s[:])


def _build_nc(with_bias=True):
    nc = bacc.Bacc("TRN2", target_bir_lowering=False, debug=False,
                   num_devices=8)

    def din(name, shape, dt=BF16):
        return nc.dram_tensor(name, shape, dt, kind="ExternalInput").ap()

    xdt = FP8 if MOE_DR and "l1" in FP8_LAYERS else BF16
    xw = din("xw", [128, D], xdt)
    xq = din("xq", [128, D], xdt)
    dt1 = FP8 if "l1" in FP8_LAYERS else BF16
    dt2 = FP8 if "l2" in FP8_LAYERS else BF16
    dt3 = FP8 if "l3" in FP8_LAYERS else BF16
    w1cb = din("w1cb", [3 * 128, 2 * NK1 * 512], dt1)
    w2cb = din("w2cb", [6 * 128, NK2 * 512], dt2)
    w3cb = din("w3cb", [10 * 128, NK2 * 512], dt3)
    w3cbl = din("w3cbl", [128, NK2 * 128], dt3)
    w1qcb = din("w1qcb", [3 * 128, 2 * NK1 * 512], dt1)
    w2qcb = din("w2qcb", [2 * 128, NK2 * 384], dt2)
    w3qcb = din("w3qcb", [5 * 128, 2 * 6 * 512], dt3)
    w3qcbl = din("w3qcbl", [128, 6 * 128], dt3)
    if with_bias:
        b1w = din("b1w", [1, H])
        b2w = din("b2w", [1, H])
        b3w = din("b3w", [1, CPAD])
        b1q = din("b1q", [1, H])
        b2q = din("b2q", [1, QCOLS])
    ones = din("ones", [1, 128])
    ident = din("ident", [128, 128])
    outw = nc.dram_tensor("outw", [128, CPAD], mybir.dt.float16,
                          kind="ExternalOutput").ap()
    outq = nc.dram_tensor("outq", [128, CPAD], mybir.dt.float16,
                          kind="ExternalOutput").ap()
    outs = nc.dram_tensor("outs", [128, 11], F32,
                          kind="ExternalOutput").ap()

    with tile.TileContext(nc) as tc:
        with tc.tile_pool(name="hp", bufs=3) as hpool, \
             tc.tile_pool(name="wp12", bufs=6) as wpool12, \
             tc.tile_pool(name="wp9", bufs=3) as wpool9, \
             tc.tile_pool(name="wp6", bufs=3) as wpool6, \
             tc.tile_pool(name="wp3", bufs=1) as wpool3, \
             tc.tile_pool(name="wp1", bufs=1) as wpool1, \
             tc.tile_pool(name="ep", bufs=2) as epool, \
             tc.tile_pool(name="sp", bufs=1) as spool, \
             tc.tile_pool(name="cp", bufs=1) as cpool, \
             tc.tile_pool(name="ps", bufs=5, space="PSUM") as pspool, \
             tc.tile_pool(name="tp", bufs=3, space="PSUM") as tppool:
            wpool = {12288: wpool12, 9216: wpool9, 6144: wpool6,
                     3072: wpool3, 768: wpool1}
            pools = (hpool, wpool, epool, spool, pspool, tppool)

            # x + consts all ride the ACT HWDGE queue so the SP (sync) queue
            # leads with the first weight piece -- the weight stream is the
            # DMA bottleneck and must start at t=0
            xw_t = cpool.tile([128, D], xdt, tag="xw")
            nc.scalar.dma_start(xw_t[:], xw)
            xq_t = cpool.tile([128, D], xdt, tag="xq")
            nc.scalar.dma_start(xq_t[:], xq)
            ones_t = cpool.tile([1, 128], BF16, tag="ones")
            nc.scalar.dma_start(ones_t[:], ones)
            ident_t = cpool.tile([128, 128], BF16, tag="ident")
            nc.scalar.dma_start(ident_t[:], ident)
            if with_bias:
                b1w_t = cpool.tile([1, H], BF16, tag="b1w")
                nc.scalar.dma_start(b1w_t[:], b1w)
                b2w_t = cpool.tile([1, H], BF16, tag="b2w")
                nc.scalar.dma_start(b2w_t[:], b2w)
                b3w_t = cpool.tile([1, CPAD], BF16, tag="b3w")
                nc.scalar.dma_start(b3w_t[:], b3w)
                b1q_t = cpool.tile([1, H], BF16, tag="b1q")
                nc.scalar.dma_start(b1q_t[:], b1q)
                b2q_t = cpool.tile([1, QCOLS], BF16, tag="b2q")
                nc.scalar.dma_start(b2q_t[:], b2q)
            else:
                b1w_t = b2w_t = b3w_t = b1q_t = b2q_t = None

            # quarter layer 1 FIRST (smallest stream prefix) so the h1
            # AllGather across the 4-core group starts as early as possible
            # and its latency hides under the whole unit's weight stream
            h1t_w = _unit_front(nc, pools, xw_t[:], w1cb, b1w_t, ones_t,
                                ident_t, "w", with_bias=with_bias)
            h1t_q = _unit_front(nc, pools, xq_t[:], w1qcb, b1q_t, ones_t,
                                ident_t, "q", with_bias=with_bias)
            h2t_w = _unit_mid(nc, pools, h1t_w, w2cb, b2w_t, ones_t, ident_t,
                              H, 512, "w", with_bias=with_bias)
            h2t_q = _unit_mid(nc, pools, h1t_q, w2qcb, b2q_t, ones_t,
                              ident_t, QCOLS, 384, "q", with_bias=with_bias)
            _l3_tail(nc, pools, h2t_w, h2t_q, w3cb, w3cbl, w3qcb, w3qcbl,
                     b3w_t, ones_t, ident_t, outw, outq, outs,
                     with_bias=with_bias)
    nc.compile()
    return nc


def _cb_pack(W, cw, layer):
    """[K, Ctot] -> per-cw-chunk column blocks [nch*128, nk*cw] where
    block row p, col k*cw + c = W[k*128 + p, j*cw + c]. In DoubleRow mode
    rows pair up per 256-super: col t*2cw + i*cw + c maps to
    row 256t + 128i + p."""
    K, Ct = W.shape
    nk, nch = K // 128, Ct // cw
    if layer in FP8_LAYERS:
        ndt = NF8
        Wr = (np.asarray(W, dtype=np.float32) * FP8_SCALE).astype(NF8)
    else:
        ndt = NBF
        Wr = np.asarray(W, dtype=NBF)
    Wr = Wr.reshape(nk, 128, Ct)
    out = np.empty((nch * 128, nk * cw), dtype=ndt)
    for j in range(nch):
        blk = Wr[:, :, j * cw:(j + 1) * cw]        # [nk, 128, cw]
        if _dr_on(layer):
            # [t, i, p, c] -> [p, t, i, c] -> cols ordered t*2cw + i*cw + c
            out[j * 128:(j + 1) * 128] = (
                blk.reshape(nk // 2, 2, 128, cw).transpose(2, 0, 1, 3)
                .reshape(128, nk * cw))
        else:
            out[j * 128:(j + 1) * 128] = (
                blk.transpose(1, 0, 2).reshape(128, nk * cw))
    return out


def _pair_fold(cb, nblocks):
    """Stack consecutive 128-row blocks side by side: [n*128, w] ->
    [(n//2)*128, 2w], so one DMA fetches two chunks' weights."""
    n128, w = cb.shape
    assert n128 == nblocks * 128 and nblocks % 2 == 0
    out = np.empty((nblocks // 2 * 128, 2 * w), dtype=cb.dtype)
    for jj in range(nblocks // 2):
        out[jj * 128:(jj + 1) * 128, :w] = cb[2 * jj * 128:(2 * jj + 1) * 128]
        out[jj * 128:(jj + 1) * 128, w:] = cb[(2 * jj + 1) * 128:
                                              (2 * jj + 2) * 128]
    return out


def _erf(v):
    try:
        from scipy.special import erf
        return erf(v)
    except ImportError:
        import math
        return np.vectorize(math.erf)(v)


def _host_expert(x_rows, W1e, b1e, W2e, b2e, W3e, b3e):
    """fp32 numpy fallback, mirroring the reference exactly."""

    def gelu(v):
        return (v * 0.5 * (1.0 + _erf(v / np.sqrt(2.0)))).astype(np.float32)

    h1 = gelu(x_rows @ W1e + b1e)
    h2 = gelu(h1 @ W2e + b2e)
    z = (h2 @ W3e + b3e).astype(np.float64)
    m = z.max(axis=1, keepdims=True)
    lse = np.log(np.exp(z - m).sum(axis=1, keepdims=True)) + m
    return (z - lse).astype(np.float32)


def kernel(x, mask_frac, W1, b1, W2, b2, W3, b3):
    global LAST_RESULTS, _NC_CACHE

    x = np.asarray(x, dtype=np.float32)
    mask_frac = np.asarray(mask_frac, dtype=np.float32)
    W1 = np.asarray(W1, dtype=np.float32)
    b1 = np.asarray(b1, dtype=np.float32)
    W2 = np.asarray(W2, dtype=np.float32)
    b2 = np.asarray(b2, dtype=np.float32)
    W3 = np.asarray(W3, dtype=np.float32)
    b3 = np.asarray(b3, dtype=np.float32)

    # host routing, mirroring the reference's fp32 arithmetic
    t = np.float32(1.0) - mask_frac
    bins = (t / np.float32(0.1)).astype(np.int32)

    with_bias = bool(b1.any() or b2.any() or b3.any())

    groups = [np.where(bins == e)[0] for e in range(E)]
    fallback = []  # (expert, sample indices) pairs computed on host
    dev_groups = []
    for e in range(10):
        idx = groups[e]
        if len(idx) > CAP:
            fallback.append((e, idx[CAP:]))
            idx = idx[:CAP]
        dev_groups.append(idx)
    if len(groups[10]):
        fallback.append((10, groups[10]))

    NX = NF8 if (MOE_DR and "l1" in FP8_LAYERS) else NBF

    def pack_x(idx):
        # [128, D] with xs[p, k*128 + n] = x[idx[n], k*128 + p]; this
        # layout is already DoubleRow-compatible (k-tile pairs sit in
        # adjacent 128-col groups)
        xt = np.zeros((128, D), dtype=NX)
        if len(idx):
            xe = x[idx].astype(NX)             # [n, D]
            xr = np.ascontiguousarray(
                xe.reshape(len(idx), NK1, 128).transpose(2, 1, 0))
            xt.reshape(128, NK1, 128)[:, :, :len(idx)] = xr
        return xt

    bsc1 = FP8_SCALE if "l1" in FP8_LAYERS else 1.0
    bsc2 = FP8_SCALE if "l2" in FP8_LAYERS else 1.0
    bsc3 = FP8_SCALE if "l3" in FP8_LAYERS else 1.0
    b3pad = np.full((1, CPAD), PAD_BIAS * bsc3, dtype=NBF)
    ones_np = np.ones((1, 128), dtype=NBF)
    ident_np = np.eye(128, dtype=NBF)

    in_maps = []
    for c in range(8):
        q = 8 if c < 4 else 9          # split expert handled by this core
        qq = c % 4                     # hidden-dim quarter index
        b3row = b3pad.copy()
        b3row[0, :C] = (b3[c] * bsc3).astype(NBF)
        w3pad = np.zeros((H, CPAD), dtype=np.float32)
        w3pad[:, :C] = W3[c]
        w3qpad = np.zeros((QCOLS, CPAD), dtype=np.float32)
        w3qpad[:, :C] = W3[q][qq * QCOLS:(qq + 1) * QCOLS]
        bias_ins = {
            "b1w": (b1[c] * bsc1).astype(NBF).reshape(1, H),
            "b2w": (b2[c] * bsc2).astype(NBF).reshape(1, H),
            "b3w": b3row,
            "b1q": (b1[q] * bsc1).astype(NBF).reshape(1, H),
            "b2q": np.ascontiguousarray(
                (b2[q][qq * QCOLS:(qq + 1) * QCOLS] * bsc2).astype(NBF)
            ).reshape(1, QCOLS),
        } if with_bias else {}
        in_maps.append({
            **bias_ins,
            "xw": pack_x(dev_groups[c]),
            "xq": pack_x(dev_groups[q]),
            "w1cb": _pair_fold(_cb_pack(W1[c], 512, "l1"), 6),
            "w2cb": _cb_pack(W2[c], 512, "l2"),
            "w3cb": _cb_pack(w3pad[:, :CMAIN], 512, "l3"),
            "w3cbl": _cb_pack(w3pad[:, CMAIN:], 128, "l3"),
            "w1qcb": _pair_fold(_cb_pack(W1[q], 512, "l1"), 6),
            "w2qcb": _cb_pack(W2[q][:, qq * QCOLS:(qq + 1) * QCOLS], 384, "l2"),
            "w3qcb": _pair_fold(_cb_pack(w3qpad[:, :CMAIN], 512, "l3"), 10),
            "w3qcbl": _cb_pack(w3qpad[:, CMAIN:], 128, "l3"),
            "ones": ones_np,
            "ident": ident_np,
        })

    if with_bias not in _NC_CACHE:
        _NC_CACHE[with_bias] = _build_nc(with_bias)
    res = run_bass_kernel_spmd(_NC_CACHE[with_bias], in_maps,
                               core_ids=list(range(8)))
    LAST_RESULTS = res

    out = np.zeros((B, C), dtype=np.float32)
    for c in range(8):
        idx = dev_groups[c]
        if len(idx):
            z = res.results[c]["outw"][:len(idx), :C].astype(np.float32)
            stot = (res.results[c]["outs"][:len(idx)]
                    .astype(np.float64).sum(axis=1, keepdims=True))
            lse = np.log(stot + (C - CPAD) * (0.0 if with_bias else 1.0))
            out[idx] = z - lse.astype(np.float32)

    # split experts: host-sum the 4 hidden-quarter partials + b3, log_softmax
    for q, cores in ((8, (0, 1, 2, 3)), (9, (4, 5, 6, 7))):
        idx = dev_groups[q]
        if not len(idx):
            continue
        zsum = np.zeros((len(idx), C), dtype=np.float64)
        for c in cores:
            zsum += res.results[c]["outq"][:len(idx), :C]
        zsum += b3[q]
        m = zsum.max(axis=1, keepdims=True)
        lse = np.log(np.exp(zsum - m).sum(axis=1, keepdims=True)) + m
        out[idx] = (zsum - lse).astype(np.float32)

    for e, idx in fallback:
        out[idx] = _host_expert(x[idx], W1[e], b1[e], W2[e], b2[e],
                                W3[e], b3[e])
    return out

